# revision 11
# baseline (speedup 1.0000x reference)
"""RBF kernel feature map: out[b, r] = exp(-||x[b] - refs[r]||^2).

Computed via the GEMM expansion on 8 NeuronCores, data-parallel over the
batch dim of x (2048 rows per core), refs replicated.

Per-core device kernel, one K=66 matmul per [128, 512] PSUM bank:
    psum[b, r] = 2*sum_d x[b,d]*refs[r,d] - r_sq[r]
    out[b, r]  = exp(psum[b, r] - x_sq[b])     (x_sq rides the per-
                                                partition ACT bias AP)

The 2x is folded into the packed x rows; r_sq is split hi/lo across two
extra fp16 K rows; x_sq is exact f32, shipped bitcast as 32 fp16 cols
inside the main input tensor and read back via an AP bitcast.  All
matmul operands are fp16 (full-rate PE at the sustained 1.2GHz clock);
the Exp activation covers a [128, 2048] 4-bank PSUM span per steady
instruction (ACT cost law measured 260ns + 0.833ns/col) and writes
bf16, halving the dominant output HBM traffic (the host upcasts).
The first and last batch tiles use two half-span activations instead:
the first starts the ACT chain ~1.2us earlier, the last lets the final
DMAs drain ~1us earlier.

Input DMA: K rows are padded to 128 DRAM rows because DMA engine spread
is partition-driven — [68, n] lands on ~4 of 16 SDMA engines, [128, n]
on all 16.  Pieces go on the single sync HWDGE ring (~280GB/s, FIFO) in
first-use order.  (SWDGE descriptor generation measured ~6us for this
pattern; HWDGE rings opened on the scalar queue tax every ACT
instruction ~0.4us — so everything rides the sync ring.)

Measured rel err vs fp64 reference ~3.6e-3 against a 2e-2 gate.

Uses bacc.Bacc (not raw bass.Bass): TRN2 instructions carry at most one
semaphore wait, and Bacc.compile()'s generate_event_semaphores pass
legalizes the multi-wait instructions Tile emits.
"""

import numpy as np

N_CORES = 8
B, D, R = 16384, 64, 2048
B_SHARD = B // N_CORES  # 2048
K = D + 2  # 64 data rows + r_sq hi/lo rows (x_sq rides the ACT bias)
KP = 128  # K padded to full partition count for 16-engine DMA spread
BT = 128  # batch rows per tile (PSUM partition dim)
RC = 512  # refs cols per matmul (one fp32 PSUM bank)
ACT_COLS = 2048  # steady Exp activation span: 4 PSUM banks
N_BT = B_SHARD // BT  # 16
XSQ = 2 * N_BT  # 32 fp16 cols holding 16 f32 -x_sq values per partition
X0 = XSQ  # x block 0 at cols [32, 160)
REFS = X0 + BT  # refs at cols [160, 2208)
XR = REFS + R  # x blocks 1..15 at cols [2208, 4128)
NC_IN = XR + B_SHARD - BT  # 4128


def _build_nc():
    from contextlib import ExitStack

    import concourse.tile as tile
    from concourse import bacc, mybir

    f16 = mybir.dt.float16
    bf16 = mybir.dt.bfloat16
    f32 = mybir.dt.float32

    nc = bacc.Bacc(None)
    inT_aug = nc.declare_dram_parameter("inT_aug", [KP, NC_IN], f16, isOutput=False)
    out = nc.declare_dram_parameter("out", [B_SHARD, R], bf16, isOutput=True)

    n_rc = R // RC

    with tile.TileContext(nc) as tc, ExitStack() as ctx:
        consts = ctx.enter_context(tc.tile_pool(name="consts", bufs=1))
        outs = ctx.enter_context(tc.tile_pool(name="outs", bufs=4))
        psums = ctx.enter_context(tc.tile_pool(name="psums", bufs=2, space="PSUM"))

        in_sb = consts.tile([KP, NC_IN], f16)
        # Two parallel input paths: the sync HWDGE ring ships the bt0-critical
        # pieces (xsq+x0, refs halves) in first-use order, while gpsimd SWDGE
        # sprays the x remainder (gates only bt>=1) across all 16 engines.
        # With 128 DRAM rows the SWDGE descriptors are 128x~4KB, so Q7
        # generation is ~1us (the old 68-row shape generated 448 tiny packets
        # over ~6us).
        for lo, hi in (
            (0, REFS),
            (REFS, REFS + 2 * RC),
            (REFS + 2 * RC, XR),
        ):
            nc.sync.dma_start(out=in_sb[:, lo:hi], in_=inT_aug[:, lo:hi])
        nc.gpsimd.dma_start(out=in_sb[:, XR:], in_=inT_aug[:, XR:])

        def lhsT(bt):
            base = X0 if bt == 0 else XR + (bt - 1) * BT
            return in_sb[:K, base : base + BT]

        for bt in range(N_BT):
            ps = psums.tile([BT, ACT_COLS], f32)
            out_sb = outs.tile([BT, R], bf16)
            for rc in range(n_rc):
                nc.tensor.matmul(
                    ps[:, rc * RC : (rc + 1) * RC],
                    lhsT=lhsT(bt),
                    rhs=in_sb[:K, REFS + rc * RC : REFS + (rc + 1) * RC],
                    start=True,
                    stop=True,
                )
            bias = in_sb[:, 2 * bt : 2 * bt + 2].bitcast(f32)
            if bt == N_BT - 1:
                # half-span ACTs let the final DMAs drain ~1us earlier
                h = ACT_COLS // 2
                for j in range(2):
                    nc.scalar.activation(
                        out_sb[:, j * h : (j + 1) * h],
                        ps[:, j * h : (j + 1) * h],
                        mybir.ActivationFunctionType.Exp,
                        bias=bias, scale=1.0,
                    )
                    nc.sync.dma_start(
                        out=out[bt * BT : (bt + 1) * BT, j * h : (j + 1) * h],
                        in_=out_sb[:, j * h : (j + 1) * h],
                    )
            else:
                nc.scalar.activation(
                    out_sb, ps, mybir.ActivationFunctionType.Exp,
                    bias=bias, scale=1.0,
                )
                nc.sync.dma_start(out=out[bt * BT : (bt + 1) * BT, :], in_=out_sb)

    nc.compile()
    return nc


def _hi_lo(v):
    """Split fp64 vector into fp16-representable hi + fp16 remainder lo."""
    hi = v.astype(np.float16)
    lo = (v - hi.astype(np.float64)).astype(np.float16)
    return hi, lo


def make_in_maps(x, refs):
    """Host-side prep: shard/transpose x, pack refs norms as extra K rows.

    The x data rows carry 2x so the K=66 contraction plus the -x_sq ACT
    bias yields 2*x.r - r_sq - x_sq = -||x - r||^2.
    """
    x = np.ascontiguousarray(x, dtype=np.float32)
    refs = np.ascontiguousarray(refs, dtype=np.float32)

    r_hi, r_lo = _hi_lo((refs.astype(np.float64) ** 2).sum(axis=1))
    x_sq = (x.astype(np.float64) ** 2).sum(axis=1)  # [B]
    xT16 = np.ascontiguousarray((2.0 * x.T).astype(np.float16))  # [D, B]
    rT16 = np.ascontiguousarray(refs.T.astype(np.float16))  # [D, R]

    in_maps = []
    for c in range(N_CORES):
        sl = slice(c * B_SHARD, (c + 1) * B_SHARD)
        inT_aug = np.zeros((KP, NC_IN), np.float16)
        xc = xT16[:, sl]
        xsq_neg = np.ascontiguousarray(
            -x_sq[sl].astype(np.float32).reshape(N_BT, BT).T
        )  # [BT, N_BT] f32; col bt = -x_sq of that block's rows
        inT_aug[:BT, :XSQ] = xsq_neg.view(np.float16)
        inT_aug[:D, X0:REFS] = xc[:, :BT]
        inT_aug[D, X0:REFS] = 1.0
        inT_aug[D + 1, X0:REFS] = 1.0
        inT_aug[:D, REFS:XR] = rT16
        inT_aug[D, REFS:XR] = -r_hi
        inT_aug[D + 1, REFS:XR] = -r_lo
        inT_aug[:D, XR:] = xc[:, BT:]
        inT_aug[D, XR:] = 1.0
        inT_aug[D + 1, XR:] = 1.0
        in_maps.append({"inT_aug": inT_aug})
    return in_maps


_NC_CACHE = None


def get_nc():
    global _NC_CACHE
    if _NC_CACHE is None:
        _NC_CACHE = _build_nc()
    return _NC_CACHE


def kernel(x, refs):
    from concourse.bass_utils import run_bass_kernel_spmd

    in_maps = make_in_maps(x, refs)
    res = run_bass_kernel_spmd(
        get_nc(), in_maps, core_ids=list(range(N_CORES))
    ).results
    return np.concatenate(
        [res[c]["out"].astype(np.float32) for c in range(N_CORES)], axis=0
    )


# revision 13
# speedup vs baseline: 1.0655x; 1.0655x over previous
"""RBF kernel feature map: out[b, r] = exp(-||x[b] - refs[r]||^2).

Computed via the GEMM expansion on 8 NeuronCores, data-parallel over the
batch dim of x (2048 rows per core), refs replicated.

Per-core device kernel, one K=66 matmul per [128, 512] PSUM bank:
    psum[b, r] = 2*sum_d x[b,d]*refs[r,d] - r_sq[r]
    out[b, r]  = exp(psum[b, r] - x_sq[b])     (x_sq rides the per-
                                                partition ACT bias AP)

The 2x is folded into the packed x rows; r_sq is split hi/lo across two
extra fp16 K rows; x_sq is exact f32, shipped bitcast as 32 fp16 cols
inside the main input tensor and read back via an AP bitcast.  All
matmul operands are fp16 (full-rate PE at the sustained 1.2GHz clock);
the Exp activation covers a [128, 2048] 4-bank PSUM span per steady
instruction (ACT cost law measured 260ns + 0.833ns/col) and writes
bf16, halving the dominant output HBM traffic (the host upcasts).
The first and last batch tiles use two half-span activations instead:
the first starts the ACT chain ~1.2us earlier, the last lets the final
DMAs drain ~1us earlier.

Input DMA: K rows are padded to 128 DRAM rows because DMA engine spread
is partition-driven — [68, n] lands on ~4 of 16 SDMA engines, [128, n]
on all 16.  Pieces go on the single sync HWDGE ring (~280GB/s, FIFO) in
first-use order.  (SWDGE descriptor generation measured ~6us for this
pattern; HWDGE rings opened on the scalar queue tax every ACT
instruction ~0.4us — so everything rides the sync ring.)

Measured rel err vs fp64 reference ~3.6e-3 against a 2e-2 gate.

Uses bacc.Bacc (not raw bass.Bass): TRN2 instructions carry at most one
semaphore wait, and Bacc.compile()'s generate_event_semaphores pass
legalizes the multi-wait instructions Tile emits.
"""

import numpy as np

N_CORES = 8
B, D, R = 16384, 64, 2048
B_SHARD = B // N_CORES  # 2048
K = D + 2  # 64 data rows + r_sq hi/lo rows (x_sq rides the ACT bias)
KP = 128  # K padded to full partition count for 16-engine DMA spread
BT = 128  # batch rows per tile (PSUM partition dim)
RC = 512  # refs cols per matmul (one fp32 PSUM bank)
ACT_COLS = 2048  # steady Exp activation span: 4 PSUM banks
N_BT = B_SHARD // BT  # 16
XSQ = 2 * N_BT  # 32 fp16 cols holding 16 f32 -x_sq values per partition
X0 = XSQ  # x block 0 at cols [32, 160)
REFS = X0 + BT  # refs at cols [160, 2208)
XR = REFS + R  # x blocks 1..15 at cols [2208, 4128)
NC_IN = XR + B_SHARD - BT  # 4128


def _build_nc():
    from contextlib import ExitStack

    import concourse.tile as tile
    from concourse import bacc, mybir

    f16 = mybir.dt.float16
    bf16 = mybir.dt.bfloat16
    f32 = mybir.dt.float32

    nc = bacc.Bacc(None)
    inT_aug = nc.declare_dram_parameter("inT_aug", [KP, NC_IN], f16, isOutput=False)
    out = nc.declare_dram_parameter("out", [B_SHARD, R], bf16, isOutput=True)

    n_rc = R // RC

    with tile.TileContext(nc) as tc, ExitStack() as ctx:
        consts = ctx.enter_context(tc.tile_pool(name="consts", bufs=1))
        outs = ctx.enter_context(tc.tile_pool(name="outs", bufs=4))
        psums = ctx.enter_context(tc.tile_pool(name="psums", bufs=2, space="PSUM"))

        in_sb = consts.tile([KP, NC_IN], f16)
        # Pieces serialize FIFO on the one sync HWDGE ring; ship in first-use
        # order so subtile deps release each matmul as its piece lands:
        # xsq+x0 | r0 r1 | r2 r3 | x1-5 | x6-10 | x11-15.  (Alternatives
        # measured worse: SWDGE descriptor generation on Q7 runs ~38ns/desc
        # and a gpsimd writer on this tile coarsens the first matmul's dep to
        # the whole transfer; HWDGE rings on the scalar queue tax every ACT
        # instruction ~0.4us.)
        for lo, hi in (
            (0, REFS),
            (REFS, REFS + 2 * RC),
            (REFS + 2 * RC, XR),
            (XR, XR + 5 * BT),
            (XR + 5 * BT, XR + 10 * BT),
            (XR + 10 * BT, NC_IN),
        ):
            nc.sync.dma_start(out=in_sb[:, lo:hi], in_=inT_aug[:, lo:hi])

        def lhsT(bt):
            base = X0 if bt == 0 else XR + (bt - 1) * BT
            return in_sb[:K, base : base + BT]

        for bt in range(N_BT):
            ps = psums.tile([BT, ACT_COLS], f32)
            out_sb = outs.tile([BT, R], bf16)
            for rc in range(n_rc):
                nc.tensor.matmul(
                    ps[:, rc * RC : (rc + 1) * RC],
                    lhsT=lhsT(bt),
                    rhs=in_sb[:K, REFS + rc * RC : REFS + (rc + 1) * RC],
                    start=True,
                    stop=True,
                )
            bias = in_sb[:, 2 * bt : 2 * bt + 2].bitcast(f32)
            if bt == N_BT - 1:
                # split-span ACTs let the final DMAs drain ~1us earlier; the
                # trailing piece is smallest so the last transfer is shortest
                for lo, hi in ((0, 3 * RC), (3 * RC, ACT_COLS)):
                    nc.scalar.activation(
                        out_sb[:, lo:hi],
                        ps[:, lo:hi],
                        mybir.ActivationFunctionType.Exp,
                        bias=bias, scale=1.0,
                    )
                    nc.sync.dma_start(
                        out=out[bt * BT : (bt + 1) * BT, lo:hi],
                        in_=out_sb[:, lo:hi],
                    )
            else:
                nc.scalar.activation(
                    out_sb, ps, mybir.ActivationFunctionType.Exp,
                    bias=bias, scale=1.0,
                )
                nc.sync.dma_start(out=out[bt * BT : (bt + 1) * BT, :], in_=out_sb)

    nc.compile()
    return nc


def _hi_lo(v):
    """Split fp64 vector into fp16-representable hi + fp16 remainder lo."""
    hi = v.astype(np.float16)
    lo = (v - hi.astype(np.float64)).astype(np.float16)
    return hi, lo


def make_in_maps(x, refs):
    """Host-side prep: shard/transpose x, pack refs norms as extra K rows.

    The x data rows carry 2x so the K=66 contraction plus the -x_sq ACT
    bias yields 2*x.r - r_sq - x_sq = -||x - r||^2.
    """
    x = np.ascontiguousarray(x, dtype=np.float32)
    refs = np.ascontiguousarray(refs, dtype=np.float32)

    r_hi, r_lo = _hi_lo((refs.astype(np.float64) ** 2).sum(axis=1))
    x_sq = (x.astype(np.float64) ** 2).sum(axis=1)  # [B]
    xT16 = np.ascontiguousarray((2.0 * x.T).astype(np.float16))  # [D, B]
    rT16 = np.ascontiguousarray(refs.T.astype(np.float16))  # [D, R]

    in_maps = []
    for c in range(N_CORES):
        sl = slice(c * B_SHARD, (c + 1) * B_SHARD)
        inT_aug = np.zeros((KP, NC_IN), np.float16)
        xc = xT16[:, sl]
        xsq_neg = np.ascontiguousarray(
            -x_sq[sl].astype(np.float32).reshape(N_BT, BT).T
        )  # [BT, N_BT] f32; col bt = -x_sq of that block's rows
        inT_aug[:BT, :XSQ] = xsq_neg.view(np.float16)
        inT_aug[:D, X0:REFS] = xc[:, :BT]
        inT_aug[D, X0:REFS] = 1.0
        inT_aug[D + 1, X0:REFS] = 1.0
        inT_aug[:D, REFS:XR] = rT16
        inT_aug[D, REFS:XR] = -r_hi
        inT_aug[D + 1, REFS:XR] = -r_lo
        inT_aug[:D, XR:] = xc[:, BT:]
        inT_aug[D, XR:] = 1.0
        inT_aug[D + 1, XR:] = 1.0
        in_maps.append({"inT_aug": inT_aug})
    return in_maps


_NC_CACHE = None


def get_nc():
    global _NC_CACHE
    if _NC_CACHE is None:
        _NC_CACHE = _build_nc()
    return _NC_CACHE


def kernel(x, refs):
    from concourse.bass_utils import run_bass_kernel_spmd

    in_maps = make_in_maps(x, refs)
    res = run_bass_kernel_spmd(
        get_nc(), in_maps, core_ids=list(range(N_CORES))
    ).results
    return np.concatenate(
        [res[c]["out"].astype(np.float32) for c in range(N_CORES)], axis=0
    )
